# revision 1
# baseline (speedup 1.0000x reference)
"""Causal single-head attention [4, 2048, 1024] on 8 TRN2 NeuronCores.

Sharding: pure SPMD, no collectives. core = 2*b + h  (b = batch, h = query
zigzag half). Each core owns 8 query tiles of 128 rows, zigzag-interleaved so
causal work is balanced: h=0 -> global q128-tiles [0,2,4,6,9,11,13,15],
h=1 -> [1,3,5,7,8,10,12,14] (both sum to 68 causal k-tile visits).

Per-core pipeline (all matmul operands bf16, PSUM f32):
  QT[f,q]   = Wq'^T x_q^T   (Wq' = Wq/32, scale folded on host; xTq fed by host)
  KT[f,k]   = Wk^T x^T      (full 2048 keys; duplicated across the pair)
  V[k,f]    = x Wv
  S^T[k,q]  = KT^T-contracted scores in transposed layout -> exp -> * causal
              mask (0/1, host-supplied data so both parities run ONE program)
  ctx[q,f]  = sum_k E[k,q] V[k,f] accumulated in PSUM; denominator via an
              extra N=1 matmul against a ones vector; divide by reciprocal.

The scores layout [k,q] makes E directly usable as the stationary operand of
the context matmul -- no transposes anywhere on device (host feeds x^T).
Softmax skips max-subtraction: scores = q.k/32 have |s| <~ 2, exp is safe.
"""

import os
import sys

os.environ.setdefault("JAX_PLATFORMS", "axon")
for _p in (
    "/root/.axon_site",
    "/root/.axon_site/_ro/trn_rl_repo",
    "/root/.axon_site/_ro/pypackages",
    "/opt/trn_rl_repo",
):
    if os.path.isdir(_p) and _p not in sys.path:
        sys.path.append(_p)

import ml_dtypes
import numpy as np

import concourse.bass as bass  # noqa: F401  (import keeps bass registered)
import concourse.tile as tile
from concourse import bacc, mybir
from concourse.bass_utils import run_bass_kernel_spmd

bf16 = ml_dtypes.bfloat16

B, S, D = 4, 2048, 1024
P = 128
NQT = S // P                  # 16 global q128 tiles per batch
N_CORES = 8
SCALE = 1.0 / 32.0            # 1/sqrt(1024)

# zigzag query-tile assignment: pair (j, 15-j), alternate between halves
GSEL = (
    [0, 2, 4, 6, 9, 11, 13, 15],   # h = 0
    [1, 3, 5, 7, 8, 10, 12, 14],   # h = 1
)
KJ = (8, 16)                   # scores k128-tile count per local q512 block
KAV = [2, 4, 6, 8, 10, 12, 14, 16]  # context k128-tile count per local q128
N_MASKS = sum(KJ)              # 24


def _masks_for(gsel: list[int]) -> np.ndarray:
    """[24,128,512] bf16 0/1 masks, scores layout [k=part, q=free]."""
    m = np.zeros((N_MASKS, P, 4 * P), dtype=np.float32)
    tri = (np.arange(P)[:, None] <= np.arange(P)[None, :]).astype(np.float32)
    mi = 0
    for J in range(2):
        for t in range(KJ[J]):
            for c in range(4):
                g = gsel[4 * J + c]
                if t < g:
                    m[mi, :, P * c:P * (c + 1)] = 1.0
                elif t == g:
                    m[mi, :, P * c:P * (c + 1)] = tri
            mi += 1
    return m.astype(bf16)


def _emit(nc, tc, reps=1):
    f32 = mybir.dt.float32
    bt = mybir.dt.bfloat16
    ND = D // P                # 8

    xt_d = nc.dram_tensor("xt", [D, S], bt, kind="ExternalInput").ap()
    xtq_d = nc.dram_tensor("xtq", [D, D], bt, kind="ExternalInput").ap()
    wq_d = nc.dram_tensor("wq", [D, D], bt, kind="ExternalInput").ap()
    wk_d = nc.dram_tensor("wk", [D, D], bt, kind="ExternalInput").ap()
    wv_d = nc.dram_tensor("wv", [D, D], bt, kind="ExternalInput").ap()
    mask_d = nc.dram_tensor("masks", [N_MASKS, P, 4 * P], bt, kind="ExternalInput").ap()
    out_d = nc.dram_tensor("out", [D, D], f32, kind="ExternalOutput").ap()

    for _rep in range(reps):
        _emit_once(nc, tc, xt_d, xtq_d, wq_d, wk_d, wv_d, mask_d, out_d)


def _emit_once(nc, tc, xt_d, xtq_d, wq_d, wk_d, wv_d, mask_d, out_d):
    f32 = mybir.dt.float32
    bt = mybir.dt.bfloat16
    ND = D // P                # 8

    with (
        tc.tile_pool(name="xtp", bufs=ND) as xtp,
        tc.tile_pool(name="ktp", bufs=ND) as ktp,
        tc.tile_pool(name="vp", bufs=S // P) as vp,
        tc.tile_pool(name="qtp", bufs=ND) as qtp,
        tc.tile_pool(name="ep", bufs=18) as ep,
        tc.tile_pool(name="mp", bufs=6) as mp,
        tc.tile_pool(name="op", bufs=4) as op,
        tc.tile_pool(name="smallp", bufs=1) as smallp,
    ):
        ones = smallp.tile([P, 1], bt, tag="ones")
        nc.vector.memset(ones[:], 1.0)

        qt = [qtp.tile([P, D], bt, tag="qt", name=f"qt{m}") for m in range(ND)]
        kt = [ktp.tile([P, S], bt, tag="kt", name=f"kt{m}") for m in range(ND)]
        vv = [vp.tile([P, D], bt, tag="v", name=f"v{k}") for k in range(S // P)]

        # ---- projections ----
        with (
            tc.tile_pool(name="wp", bufs=10) as wp,
            tc.tile_pool(name="xqp", bufs=ND) as xqp,
            tc.tile_pool(name="pp", bufs=4, space="PSUM") as pp,
        ):
            # DMA issue order matters: the first matmul group needs wq+xtq, so
            # issue those first; xt (4MB) would otherwise hog the queue and
            # stall the PE for ~20us at kernel start.
            xtq = []
            wqt = []
            for di in range(ND):
                t = wp.tile([P, D], bt, tag="w", name=f"wq{di}")
                nc.sync.dma_start(t[:], wq_d[P * di:P * (di + 1), :])
                wqt.append(t)
                t2 = xqp.tile([P, D], bt, tag="xq", name=f"xtq{di}")
                nc.sync.dma_start(t2[:], xtq_d[P * di:P * (di + 1), :])
                xtq.append(t2)

            xt = []
            for di in range(ND):
                t = xtp.tile([P, S], bt, tag="xt", name=f"xt{di}")
                nc.sync.dma_start(t[:], xt_d[P * di:P * (di + 1), :])
                xt.append(t)
            for m in range(ND):
                for qb in range(2):
                    ps = pp.tile([P, 512], f32, tag="ps", name="psq")
                    for di in range(ND):
                        nc.tensor.matmul(
                            ps[:],
                            wqt[di][:, P * m:P * (m + 1)],
                            xtq[di][:, 512 * qb:512 * (qb + 1)],
                            start=(di == 0), stop=(di == ND - 1),
                        )
                    nc.vector.tensor_copy(qt[m][:, 512 * qb:512 * (qb + 1)], ps[:])

            # K^T[f, k] = sum_d Wk[d, f] xT[d, k]
            wkt = []
            for di in range(ND):
                t = wp.tile([P, D], bt, tag="w", name=f"wk{di}")
                nc.sync.dma_start(t[:], wk_d[P * di:P * (di + 1), :])
                wkt.append(t)
            for m in range(ND):
                for kb in range(S // 512):
                    ps = pp.tile([P, 512], f32, tag="ps", name="psk")
                    for di in range(ND):
                        nc.tensor.matmul(
                            ps[:],
                            wkt[di][:, P * m:P * (m + 1)],
                            xt[di][:, 512 * kb:512 * (kb + 1)],
                            start=(di == 0), stop=(di == ND - 1),
                        )
                    nc.vector.tensor_copy(kt[m][:, 512 * kb:512 * (kb + 1)], ps[:])

            # V[k, f] = sum_d xT[d, k] Wv[d, f]
            wvt = []
            for di in range(ND):
                t = wp.tile([P, D], bt, tag="w", name=f"wv{di}")
                nc.sync.dma_start(t[:], wv_d[P * di:P * (di + 1), :])
                wvt.append(t)
            for k in range(S // P):
                for fh in range(2):
                    ps = pp.tile([P, 512], f32, tag="ps", name="psv")
                    for di in range(ND):
                        nc.tensor.matmul(
                            ps[:],
                            xt[di][:, P * k:P * (k + 1)],
                            wvt[di][:, 512 * fh:512 * (fh + 1)],
                            start=(di == 0), stop=(di == ND - 1),
                        )
                    nc.vector.tensor_copy(vv[k][:, 512 * fh:512 * (fh + 1)], ps[:])

        # ---- attention ----
        with (
            tc.tile_pool(name="sp", bufs=2, space="PSUM") as sp,
            tc.tile_pool(name="cp", bufs=2, space="PSUM") as cp,
            tc.tile_pool(name="zp", bufs=2, space="PSUM") as zp,
            tc.tile_pool(name="rp", bufs=3) as rp,
        ):
            mi = 0
            for J in range(2):
                ee = []
                for t in range(KJ[J]):
                    ps = sp.tile([P, 512], f32, tag="sc", name="sc")
                    for fi in range(ND):
                        nc.tensor.matmul(
                            ps[:],
                            kt[fi][:, P * t:P * (t + 1)],
                            qt[fi][:, 512 * J:512 * (J + 1)],
                            start=(fi == 0), stop=(fi == ND - 1),
                        )
                    e = ep.tile([P, 512], bt, tag="e", name=f"e{J}_{t}")
                    nc.scalar.activation(e[:], ps[:], mybir.ActivationFunctionType.Exp)
                    mt = mp.tile([P, 512], bt, tag="m", name="mt")
                    nc.sync.dma_start(mt[:], mask_d[mi, :, :])
                    nc.vector.tensor_mul(e[:], e[:], mt[:])
                    ee.append(e)
                    mi += 1
                for c in range(4):
                    j = 4 * J + c
                    n = KAV[j]
                    ctx = cp.tile([P, D], f32, tag="ctx", name="ctx")
                    sm = zp.tile([P, 1], f32, tag="sm", name="sm")
                    for t in range(n):
                        lhs = ee[t][:, P * c:P * (c + 1)]
                        nc.tensor.matmul(ctx[:, 0:512], lhs, vv[t][:, 0:512],
                                         start=(t == 0), stop=(t == n - 1))
                        nc.tensor.matmul(ctx[:, 512:1024], lhs, vv[t][:, 512:1024],
                                         start=(t == 0), stop=(t == n - 1))
                        nc.tensor.matmul(sm[:], lhs, ones[:],
                                         start=(t == 0), stop=(t == n - 1))
                    rc = rp.tile([P, 1], f32, tag="rc", name="rc")
                    nc.vector.reciprocal(rc[:], sm[:])
                    o = op.tile([P, D], f32, tag="o", name="o")
                    nc.vector.tensor_scalar_mul(o[:], ctx[:], rc[:])
                    nc.sync.dma_start(out_d[P * j:P * (j + 1), :], o[:])


_CACHE = {}


def _build(reps=1):
    nc = bacc.Bacc(
        "TRN2", target_bir_lowering=False, debug=False,
        enable_asserts=False, num_devices=N_CORES,
    )
    with tile.TileContext(nc) as tc:
        _emit(nc, tc, reps=reps)
    nc.compile()
    return nc


def build_in_maps(x, W_query, W_key, W_value):
    wq = (np.asarray(W_query, np.float32) * SCALE).astype(bf16)
    wk = np.asarray(W_key, np.float32).astype(bf16)
    wv = np.asarray(W_value, np.float32).astype(bf16)
    masks = [_masks_for(GSEL[0]), _masks_for(GSEL[1])]
    in_maps = []
    for core in range(N_CORES):
        b, h = divmod(core, 2)
        xb = np.asarray(x[b], np.float32)
        qrows = np.concatenate([np.arange(P * g, P * (g + 1)) for g in GSEL[h]])
        in_maps.append({
            "xt": np.ascontiguousarray(xb.T).astype(bf16),
            "xtq": np.ascontiguousarray(xb[qrows].T).astype(bf16),
            "wq": wq, "wk": wk, "wv": wv,
            "masks": masks[h],
        })
    return in_maps


def assemble_out(results) -> np.ndarray:
    out = np.empty((B, S, D), dtype=np.float32)
    for core in range(N_CORES):
        b, h = divmod(core, 2)
        r = results[core]["out"]
        for j, g in enumerate(GSEL[h]):
            out[b, P * g:P * (g + 1), :] = r[P * j:P * (j + 1), :]
    return out


def kernel(x, W_query, W_key, W_value):
    if "nc" not in _CACHE:
        _CACHE["nc"] = _build()
    nc = _CACHE["nc"]
    in_maps = build_in_maps(x, W_query, W_key, W_value)
    r = run_bass_kernel_spmd(nc, in_maps, core_ids=list(range(N_CORES)))
    return assemble_out(r.results)


if __name__ == "__main__":
    rng = np.random.default_rng(0)
    x = rng.standard_normal((B, S, D), dtype=np.float32)
    bound = 1.0 / np.sqrt(D)
    wq = rng.uniform(-bound, bound, (D, D)).astype(np.float32)
    wk = rng.uniform(-bound, bound, (D, D)).astype(np.float32)
    wv = rng.uniform(-bound, bound, (D, D)).astype(np.float32)
    o = kernel(x, wq, wk, wv)
    print("out", o.shape, o.dtype, float(np.abs(o).max()))



# revision 2
# speedup vs baseline: 1.8921x; 1.8921x over previous
"""Causal single-head attention [4, 2048, 1024] on 8 TRN2 NeuronCores.

Sharding: pure SPMD, no collectives. core = 2*b + h  (b = batch, h = query
zigzag half). Each core owns 8 query tiles of 128 rows, zigzag-interleaved so
causal work is balanced: h=0 -> global q128-tiles [0,2,4,6,9,11,13,15],
h=1 -> [1,3,5,7,8,10,12,14] (both sum to 68 causal k-tile visits).

v2: Q/K projections and the QK^T scores run as fp8 e4m3 DoubleRow matmuls
(2 contraction subtiles per instruction at 2x rate); V projection and the
E.V context matmuls stay bf16 for accuracy (first rows of the causal output
are near-copies of V rows, so V error is not averaged down).

Scaling: W_q8 = fp8(Wq*32), W_k8 = fp8(Wk*32); QT/KT PSUM results (=32*q)
are copied to fp8 unscaled (std ~18, well inside e4m3 range); logits are
recovered inside the exp activation with scale 2^-15 = 1/(32*32*32) (the
last 32 = 1/sqrt(d)).

Causal masking without the 1.5MB/core mask stream: for context chunk j only
k-tiles KAV[j]-2 and KAV[j]-1 can straddle the diagonal; each gets one
[128,128] data-driven mask (ones/tri or tri/zeros depending on the core's
zigzag parity), so one program serves both parities. All earlier k-tiles are
fully below the diagonal and need no masking.
"""

import os
import sys

os.environ.setdefault("JAX_PLATFORMS", "axon")
for _p in (
    "/root/.axon_site",
    "/root/.axon_site/_ro/trn_rl_repo",
    "/root/.axon_site/_ro/pypackages",
    "/opt/trn_rl_repo",
):
    if os.path.isdir(_p) and _p not in sys.path:
        sys.path.append(_p)

import ml_dtypes
import numpy as np

import concourse.bass as bass  # noqa: F401  (import keeps bass registered)
import concourse.tile as tile
from concourse import bacc, mybir
from concourse.bass_utils import run_bass_kernel_spmd

bf16 = ml_dtypes.bfloat16
f8 = ml_dtypes.float8_e4m3

B, S, D = 4, 2048, 1024
P = 128
NQT = S // P                  # 16 global q128 tiles per batch
N_CORES = 8
W8SCALE = 32.0
EXP_SCALE = 1.0 / (W8SCALE * W8SCALE * 32.0)   # logits = psum * EXP_SCALE

# zigzag query-tile assignment: pair (j, 15-j), alternate between halves
GSEL = (
    [0, 2, 4, 6, 9, 11, 13, 15],   # h = 0
    [1, 3, 5, 7, 8, 10, 12, 14],   # h = 1
)
KJ = (8, 16)                   # scores k128-tile count per local q512 block
KAV = [2, 4, 6, 8, 10, 12, 14, 16]  # context k128-tile count per local q128
NPAIR = D // (2 * P)           # 4 contraction pair-tiles over d/f


def _emit(nc, tc, reps=1):
    f32 = mybir.dt.float32
    bt = mybir.dt.bfloat16
    e4 = mybir.dt.float8e4

    xt8_d = nc.dram_tensor("xt8", [NPAIR, P, 2, S], e4, kind="ExternalInput").ap()
    xtq8_d = nc.dram_tensor("xtq8", [NPAIR, P, 2, D], e4, kind="ExternalInput").ap()
    wq8_d = nc.dram_tensor("wq8", [NPAIR, P, 2, D], e4, kind="ExternalInput").ap()
    wk8_d = nc.dram_tensor("wk8", [NPAIR, P, 2, D], e4, kind="ExternalInput").ap()
    xt_d = nc.dram_tensor("xt", [D, S], bt, kind="ExternalInput").ap()
    wv_d = nc.dram_tensor("wv", [D, D], bt, kind="ExternalInput").ap()
    mask_d = nc.dram_tensor("masks", [P, 16, P], bt, kind="ExternalInput").ap()
    out_d = nc.dram_tensor("out", [D, D], bt, kind="ExternalOutput").ap()

    for _rep in range(reps):
        _emit_once(nc, tc, xt8_d, xtq8_d, wq8_d, wk8_d, xt_d, wv_d, mask_d, out_d)


def _emit_once(nc, tc, xt8_d, xtq8_d, wq8_d, wk8_d, xt_d, wv_d, mask_d, out_d):
    f32 = mybir.dt.float32
    bt = mybir.dt.bfloat16
    e4 = mybir.dt.float8e4
    ND = D // P                # 8
    DR = mybir.MatmulPerfMode.DoubleRow

    with (
        tc.tile_pool(name="qp", bufs=NPAIR) as qp,
        tc.tile_pool(name="kp", bufs=NPAIR) as kp,
        tc.tile_pool(name="vp", bufs=S // P) as vp,
        tc.tile_pool(name="ep", bufs=18) as ep,
        tc.tile_pool(name="op", bufs=4) as op,
        tc.tile_pool(name="smallp", bufs=1) as smallp,
        tc.tile_pool(name="maskp", bufs=1) as maskp,
    ):
        ones = smallp.tile([P, 1], bt, tag="ones")
        nc.vector.memset(ones[:], 1.0)
        masks = maskp.tile([P, 16, P], bt, tag="masks")

        qt8 = [qp.tile([P, 2, D], e4, tag="qt", name=f"qt{i}") for i in range(NPAIR)]
        kt8 = [kp.tile([P, 2, S], e4, tag="kt", name=f"kt{i}") for i in range(NPAIR)]
        vv = [vp.tile([P, D], bt, tag="v", name=f"v{k}") for k in range(S // P)]

        # ---- projections ----
        with (
            tc.tile_pool(name="wp", bufs=2 * NPAIR + ND) as wp,
            tc.tile_pool(name="xp", bufs=2 * NPAIR + ND) as xp,
            tc.tile_pool(name="pp", bufs=4, space="PSUM") as pp,
        ):
            # DMA issue order matters: the first matmul group needs wq8+xtq8.
            wq8t, xtq8t = [], []
            for i in range(NPAIR):
                t = wp.tile([P, 2, D], e4, tag="w", name=f"wq8{i}")
                nc.sync.dma_start(t[:], wq8_d[i])
                wq8t.append(t)
                t2 = xp.tile([P, 2, D], e4, tag="x", name=f"xtq8{i}")
                nc.sync.dma_start(t2[:], xtq8_d[i])
                xtq8t.append(t2)
            wk8t, xt8t = [], []
            for i in range(NPAIR):
                t = wp.tile([P, 2, D], e4, tag="w", name=f"wk8{i}")
                nc.sync.dma_start(t[:], wk8_d[i])
                wk8t.append(t)
                t2 = xp.tile([P, 2, S], e4, tag="x", name=f"xt8{i}")
                nc.sync.dma_start(t2[:], xt8_d[i])
                xt8t.append(t2)
            xtt, wvt = [], []
            for di in range(ND):
                t = xp.tile([P, S], bt, tag="x", name=f"xt{di}")
                nc.sync.dma_start(t[:], xt_d[P * di:P * (di + 1), :])
                xtt.append(t)
                t2 = wp.tile([P, D], bt, tag="w", name=f"wv{di}")
                nc.sync.dma_start(t2[:], wv_d[P * di:P * (di + 1), :])
                wvt.append(t2)
            nc.sync.dma_start(masks[:], mask_d[:])

            # QT[f, q] (x32): fp8 DoubleRow over d-pairs
            for m in range(ND):
                for qb in range(2):
                    ps = pp.tile([P, 512], f32, tag="ps", name="psq")
                    for i in range(NPAIR):
                        nc.tensor.matmul(
                            ps[:],
                            wq8t[i][:, :, P * m:P * (m + 1)],
                            xtq8t[i][:, :, 512 * qb:512 * (qb + 1)],
                            start=(i == 0), stop=(i == NPAIR - 1),
                            perf_mode=DR,
                        )
                    nc.vector.tensor_copy(
                        qt8[m // 2][:, m % 2, 512 * qb:512 * (qb + 1)], ps[:])

            # KT[f, k] (x32)
            for m in range(ND):
                for kb in range(S // 512):
                    ps = pp.tile([P, 512], f32, tag="ps", name="psk")
                    for i in range(NPAIR):
                        nc.tensor.matmul(
                            ps[:],
                            wk8t[i][:, :, P * m:P * (m + 1)],
                            xt8t[i][:, :, 512 * kb:512 * (kb + 1)],
                            start=(i == 0), stop=(i == NPAIR - 1),
                            perf_mode=DR,
                        )
                    nc.vector.tensor_copy(
                        kt8[m // 2][:, m % 2, 512 * kb:512 * (kb + 1)], ps[:])

            # V[k, f] = sum_d xT[d, k] Wv[d, f]  (bf16)
            for k in range(S // P):
                for fh in range(2):
                    ps = pp.tile([P, 512], f32, tag="ps", name="psv")
                    for di in range(ND):
                        nc.tensor.matmul(
                            ps[:],
                            xtt[di][:, P * k:P * (k + 1)],
                            wvt[di][:, 512 * fh:512 * (fh + 1)],
                            start=(di == 0), stop=(di == ND - 1),
                        )
                    nc.vector.tensor_copy(vv[k][:, 512 * fh:512 * (fh + 1)], ps[:])

        # ---- attention ----
        with (
            tc.tile_pool(name="sp", bufs=2, space="PSUM") as sp,
            tc.tile_pool(name="cp", bufs=2, space="PSUM") as cp,
            tc.tile_pool(name="zp", bufs=2, space="PSUM") as zp,
            tc.tile_pool(name="rp", bufs=3) as rp,
        ):
            for J in range(2):
                ee = []
                for t in range(KJ[J]):
                    ps = sp.tile([P, 512], f32, tag="sc", name="sc")
                    for i in range(NPAIR):
                        nc.tensor.matmul(
                            ps[:],
                            kt8[i][:, :, P * t:P * (t + 1)],
                            qt8[i][:, :, 512 * J:512 * (J + 1)],
                            start=(i == 0), stop=(i == NPAIR - 1),
                            perf_mode=DR,
                        )
                    e = ep.tile([P, 512], bt, tag="e", name=f"e{J}_{t}")
                    nc.scalar.activation(
                        e[:], ps[:], mybir.ActivationFunctionType.Exp,
                        scale=EXP_SCALE)
                    for c in range(4):
                        j = 4 * J + c
                        if t == KAV[j] - 2:
                            nc.vector.tensor_mul(
                                e[:, P * c:P * (c + 1)],
                                e[:, P * c:P * (c + 1)], masks[:, 2 * j, :])
                        elif t == KAV[j] - 1:
                            nc.vector.tensor_mul(
                                e[:, P * c:P * (c + 1)],
                                e[:, P * c:P * (c + 1)], masks[:, 2 * j + 1, :])
                    ee.append(e)
                for c in range(4):
                    j = 4 * J + c
                    n = KAV[j]
                    ctx = cp.tile([P, D], f32, tag="ctx", name="ctx")
                    sm = zp.tile([P, 1], f32, tag="sm", name="sm")
                    for t in range(n):
                        lhs = ee[t][:, P * c:P * (c + 1)]
                        nc.tensor.matmul(ctx[:, 0:512], lhs, vv[t][:, 0:512],
                                         start=(t == 0), stop=(t == n - 1))
                        nc.tensor.matmul(ctx[:, 512:1024], lhs, vv[t][:, 512:1024],
                                         start=(t == 0), stop=(t == n - 1))
                        nc.tensor.matmul(sm[:], lhs, ones[:],
                                         start=(t == 0), stop=(t == n - 1))
                    rc = rp.tile([P, 1], f32, tag="rc", name="rc")
                    nc.vector.reciprocal(rc[:], sm[:])
                    o = op.tile([P, D], bt, tag="o", name="o")
                    nc.vector.tensor_scalar_mul(o[:], ctx[:], rc[:])
                    nc.sync.dma_start(out_d[P * j:P * (j + 1), :], o[:])


_CACHE = {}


def _build(reps=1):
    nc = bacc.Bacc(
        "TRN2", target_bir_lowering=False, debug=False,
        enable_asserts=False, num_devices=N_CORES,
    )
    with tile.TileContext(nc) as tc:
        _emit(nc, tc, reps=reps)
    nc.compile()
    return nc


def _pack_pairs(m: np.ndarray) -> np.ndarray:
    """[1024, N] f32 -> [4, 128, 2, N] fp8, d = 256*i + 128*j + p."""
    n = m.shape[1]
    return np.ascontiguousarray(
        m.reshape(NPAIR, 2, P, n).transpose(0, 2, 1, 3)).astype(f8)


def _masks_for(h: int) -> np.ndarray:
    """[128, 16, 128] bf16; chunk j slots (2j, 2j+1) = masks for k-tiles
    KAV[j]-2 and KAV[j]-1. Scores layout [k=part, q=free]: allow k <= q."""
    tri = (np.arange(P)[:, None] <= np.arange(P)[None, :]).astype(np.float32)
    m = np.zeros((P, 16, P), dtype=np.float32)
    for j in range(8):
        g = GSEL[h][j]
        if g == KAV[j] - 1:
            m[:, 2 * j, :] = 1.0
            m[:, 2 * j + 1, :] = tri
        else:
            assert g == KAV[j] - 2
            m[:, 2 * j, :] = tri
            m[:, 2 * j + 1, :] = 0.0
    return m.astype(bf16)


def build_in_maps(x, W_query, W_key, W_value):
    wq8 = _pack_pairs(np.asarray(W_query, np.float32) * W8SCALE)
    wk8 = _pack_pairs(np.asarray(W_key, np.float32) * W8SCALE)
    wv = np.asarray(W_value, np.float32).astype(bf16)
    masks = [_masks_for(0), _masks_for(1)]
    in_maps = []
    for core in range(N_CORES):
        b, h = divmod(core, 2)
        xb = np.asarray(x[b], np.float32)
        xbt = np.ascontiguousarray(xb.T)               # [1024, 2048]
        qrows = np.concatenate([np.arange(P * g, P * (g + 1)) for g in GSEL[h]])
        in_maps.append({
            "xt8": _pack_pairs(xbt),
            "xtq8": _pack_pairs(np.ascontiguousarray(xb[qrows].T)),
            "wq8": wq8, "wk8": wk8,
            "xt": xbt.astype(bf16),
            "wv": wv,
            "masks": masks[h],
        })
    return in_maps


def assemble_out(results) -> np.ndarray:
    out = np.empty((B, S, D), dtype=np.float32)
    for core in range(N_CORES):
        b, h = divmod(core, 2)
        r = np.asarray(results[core]["out"], dtype=np.float32)
        for j, g in enumerate(GSEL[h]):
            out[b, P * g:P * (g + 1), :] = r[P * j:P * (j + 1), :]
    return out


def kernel(x, W_query, W_key, W_value):
    if "nc" not in _CACHE:
        _CACHE["nc"] = _build()
    nc = _CACHE["nc"]
    in_maps = build_in_maps(x, W_query, W_key, W_value)
    r = run_bass_kernel_spmd(nc, in_maps, core_ids=list(range(N_CORES)))
    return assemble_out(r.results)


if __name__ == "__main__":
    rng = np.random.default_rng(0)
    x = rng.standard_normal((B, S, D), dtype=np.float32)
    bound = 1.0 / np.sqrt(D)
    wq = rng.uniform(-bound, bound, (D, D)).astype(np.float32)
    wk = rng.uniform(-bound, bound, (D, D)).astype(np.float32)
    wv = rng.uniform(-bound, bound, (D, D)).astype(np.float32)
    o = kernel(x, wq, wk, wv)
    print("out", o.shape, o.dtype, float(np.abs(o).max()))


# revision 4
# speedup vs baseline: 2.6275x; 1.3887x over previous
"""Causal single-head attention [4, 2048, 1024] on 8 TRN2 NeuronCores.

Sharding: pure SPMD, no collectives. core = 2*b + h  (b = batch, h = query
zigzag half). Each core owns 8 query tiles of 128 rows, zigzag-interleaved so
causal work is balanced: h=0 -> global q128-tiles [0,2,4,6,9,11,13,15],
h=1 -> [1,3,5,7,8,10,12,14] (both sum to 68 causal k-tile visits).

v3: everything except the first two k-tiles of the V/context path runs as
fp8 e4m3 DoubleRow matmuls (2 contraction subtiles per instruction at 2x
rate): Q/K projections, QK^T scores, most of the V projection, and most of
the E.V context accumulation. k-tiles 0-1 stay bf16 because the first query
tile's outputs are near-copies of single V rows (no error averaging), and
they dominate the max-relative-error metric. E's fp8 quantization cancels
through the softmax: the denominator is summed from the same quantized E
values the numerator uses.

Scaling: W_q8 = fp8(Wq*32), W_k8 = fp8(Wk*32); QT/KT PSUM results (=32*q)
are copied to fp8 unscaled (std ~18, inside e4m3 range); logits are
recovered inside the exp activation with scale 2^-15 = 1/(32*32*32) (the
last 32 = 1/sqrt(d)). Wv8 = fp8(Wv) unscaled: all |Wv|<=1/32 sit in e4m3's
2^-6 binade/subnormal range whose fixed ~2^-10 abs step matches the scaled
variant's top-binade error, so no descaling pass is needed.

Causal masking without a mask stream: for context chunk j only k-tiles
KAV[j]-2 and KAV[j]-1 can straddle the diagonal; each gets one [128,128]
data-driven mask (ones/tri or tri/zeros depending on zigzag parity), so one
program serves both parities.
"""

import os
import sys

os.environ.setdefault("JAX_PLATFORMS", "axon")
for _p in (
    "/root/.axon_site",
    "/root/.axon_site/_ro/trn_rl_repo",
    "/root/.axon_site/_ro/pypackages",
    "/opt/trn_rl_repo",
):
    if os.path.isdir(_p) and _p not in sys.path:
        sys.path.append(_p)

import ml_dtypes
import numpy as np

import concourse.bass as bass  # noqa: F401  (import keeps bass registered)
import concourse.tile as tile
from concourse import bacc, mybir
from concourse.bass_utils import run_bass_kernel_spmd

bf16 = ml_dtypes.bfloat16
f8 = ml_dtypes.float8_e4m3

B, S, D = 4, 2048, 1024
P = 128
NQT = S // P                  # 16 global q128 tiles per batch
N_CORES = 8
W8SCALE = 32.0
EXP_SCALE = 1.0 / (W8SCALE * W8SCALE * 32.0)   # logits = psum * EXP_SCALE
NBF = 2                        # k128-tiles kept bf16 in the V/context path

# zigzag query-tile assignment: pair (j, 15-j), alternate between halves
GSEL = (
    [0, 2, 4, 6, 9, 11, 13, 15],   # h = 0
    [1, 3, 5, 7, 8, 10, 12, 14],   # h = 1
)
KJ = (8, 16)                   # scores k128-tile count per local q512 block
KAV = [2, 4, 6, 8, 10, 12, 14, 16]  # context k128-tile count per local q128
NPAIR = D // (2 * P)           # 4 contraction pair-tiles over d/f


def _emit(nc, tc, reps=1):
    bt = mybir.dt.bfloat16
    e4 = mybir.dt.float8e4

    xt8_d = nc.dram_tensor("xt8", [NPAIR, P, 2, S], e4, kind="ExternalInput").ap()
    xtq8_d = nc.dram_tensor("xtq8", [NPAIR, P, 2, D], e4, kind="ExternalInput").ap()
    wq8_d = nc.dram_tensor("wq8", [NPAIR, P, 2, D], e4, kind="ExternalInput").ap()
    wk8_d = nc.dram_tensor("wk8", [NPAIR, P, 2, D], e4, kind="ExternalInput").ap()
    wv8_d = nc.dram_tensor("wv8", [NPAIR, P, 2, D], e4, kind="ExternalInput").ap()
    xt_d = nc.dram_tensor("xt", [D, NBF * P], bt, kind="ExternalInput").ap()
    wv_d = nc.dram_tensor("wv", [D, D], bt, kind="ExternalInput").ap()
    mask_d = nc.dram_tensor("masks", [P, 16, P], bt, kind="ExternalInput").ap()
    out_d = nc.dram_tensor("out", [D, D], bt, kind="ExternalOutput").ap()

    for _rep in range(reps):
        _emit_once(nc, tc, xt8_d, xtq8_d, wq8_d, wk8_d, wv8_d, xt_d, wv_d,
                   mask_d, out_d)


def _emit_once(nc, tc, xt8_d, xtq8_d, wq8_d, wk8_d, wv8_d, xt_d, wv_d,
               mask_d, out_d):
    f32 = mybir.dt.float32
    bt = mybir.dt.bfloat16
    e4 = mybir.dt.float8e4
    ND = D // P                # 8
    DR = mybir.MatmulPerfMode.DoubleRow

    with (
        tc.tile_pool(name="qp", bufs=NPAIR) as qp,
        tc.tile_pool(name="kp", bufs=NPAIR) as kp,
        tc.tile_pool(name="vbp", bufs=NBF) as vbp,
        tc.tile_pool(name="vpp", bufs=(S // P - NBF) // 2) as vpp,
        tc.tile_pool(name="ebp", bufs=5) as ebp,
        tc.tile_pool(name="epp", bufs=9) as epp,
        tc.tile_pool(name="op", bufs=4) as op,
        tc.tile_pool(name="smallp", bufs=2) as smallp,
        tc.tile_pool(name="maskp", bufs=1) as maskp,
    ):
        ones = smallp.tile([P, 1], bt, tag="ones")
        nc.vector.memset(ones[:], 1.0)
        ones8 = smallp.tile([P, 2, 1], e4, tag="ones8")
        nc.vector.memset(ones8[:], 1.0)
        masks = maskp.tile([P, 16, P], bt, tag="masks")

        qt8 = [qp.tile([P, 2, D], e4, tag="qt", name=f"qt{i}") for i in range(NPAIR)]
        kt8 = [kp.tile([P, 2, S], e4, tag="kt", name=f"kt{i}") for i in range(NPAIR)]
        vvb = [vbp.tile([P, D], bt, tag="vb", name=f"vb{k}") for k in range(NBF)]
        # pair p holds k-tiles 2p, 2p+1 (p >= 1; tiles 0,1 are the bf16 vvb)
        vvp = [None] + [vpp.tile([P, 2, D], e4, tag="vp", name=f"vp{p}")
                        for p in range(1, S // (2 * P))]

        # ---- projections ----
        with (
            tc.tile_pool(name="wp", bufs=3 * NPAIR + ND) as wp,
            tc.tile_pool(name="xp", bufs=2 * NPAIR + ND) as xp,
            tc.tile_pool(name="pp", bufs=6, space="PSUM") as pp,
        ):
            # DMA issue order matters: the first matmul group needs wq8+xtq8.
            wq8t, xtq8t = [], []
            for i in range(NPAIR):
                t = wp.tile([P, 2, D], e4, tag="w", name=f"wq8{i}")
                nc.sync.dma_start(t[:], wq8_d[i])
                wq8t.append(t)
                t2 = xp.tile([P, 2, D], e4, tag="x", name=f"xtq8{i}")
                nc.sync.dma_start(t2[:], xtq8_d[i])
                xtq8t.append(t2)
            wk8t, xt8t = [], []
            for i in range(NPAIR):
                t = wp.tile([P, 2, D], e4, tag="w", name=f"wk8{i}")
                nc.sync.dma_start(t[:], wk8_d[i])
                wk8t.append(t)
                t2 = xp.tile([P, 2, S], e4, tag="x", name=f"xt8{i}")
                nc.sync.dma_start(t2[:], xt8_d[i])
                xt8t.append(t2)
            wv8t = []
            for i in range(NPAIR):
                t = wp.tile([P, 2, D], e4, tag="w", name=f"wv8{i}")
                nc.sync.dma_start(t[:], wv8_d[i])
                wv8t.append(t)
            xtt, wvt = [], []
            for di in range(ND):
                t = xp.tile([P, NBF * P], bt, tag="x", name=f"xt{di}")
                nc.sync.dma_start(t[:], xt_d[P * di:P * (di + 1), :])
                xtt.append(t)
                t2 = wp.tile([P, D], bt, tag="w", name=f"wv{di}")
                nc.sync.dma_start(t2[:], wv_d[P * di:P * (di + 1), :])
                wvt.append(t2)
            nc.sync.dma_start(masks[:], mask_d[:])

            # QT[f, q] (x32): fp8 DoubleRow over d-pairs
            for m in range(ND):
                for qb in range(2):
                    ps = pp.tile([P, 512], f32, tag="ps", name="psq")
                    for i in range(NPAIR):
                        nc.tensor.matmul(
                            ps[:],
                            wq8t[i][:, :, P * m:P * (m + 1)],
                            xtq8t[i][:, :, 512 * qb:512 * (qb + 1)],
                            start=(i == 0), stop=(i == NPAIR - 1),
                            perf_mode=DR,
                        )
                    nc.vector.tensor_copy(
                        qt8[m // 2][:, m % 2, 512 * qb:512 * (qb + 1)], ps[:])

            # KT[f, k] (x32)
            for m in range(ND):
                for kb in range(S // 512):
                    ps = pp.tile([P, 512], f32, tag="ps", name="psk")
                    for i in range(NPAIR):
                        nc.tensor.matmul(
                            ps[:],
                            wk8t[i][:, :, P * m:P * (m + 1)],
                            xt8t[i][:, :, 512 * kb:512 * (kb + 1)],
                            start=(i == 0), stop=(i == NPAIR - 1),
                            perf_mode=DR,
                        )
                    nc.vector.tensor_copy(
                        kt8[m // 2][:, m % 2, 512 * kb:512 * (kb + 1)], ps[:])

            # V[k, f] = sum_d xT[d, k] Wv[d, f]
            # k-tiles 0..NBF-1 bf16; the rest fp8 DoubleRow from xt8/wv8.
            for k in range(S // P):
                for fh in range(2):
                    ps = pp.tile([P, 512], f32, tag="ps", name="psv")
                    if k < NBF:
                        for di in range(ND):
                            nc.tensor.matmul(
                                ps[:],
                                xtt[di][:, P * k:P * (k + 1)],
                                wvt[di][:, 512 * fh:512 * (fh + 1)],
                                start=(di == 0), stop=(di == ND - 1),
                            )
                        nc.vector.tensor_copy(
                            vvb[k][:, 512 * fh:512 * (fh + 1)], ps[:])
                    else:
                        for i in range(NPAIR):
                            nc.tensor.matmul(
                                ps[:],
                                xt8t[i][:, :, P * k:P * (k + 1)],
                                wv8t[i][:, :, 512 * fh:512 * (fh + 1)],
                                start=(i == 0), stop=(i == NPAIR - 1),
                                perf_mode=DR,
                            )
                        nc.vector.tensor_copy(
                            vvp[k // 2][:, k % 2, 512 * fh:512 * (fh + 1)], ps[:])

        # ---- attention ----
        with (
            tc.tile_pool(name="sp", bufs=2, space="PSUM") as sp,
            tc.tile_pool(name="cp", bufs=2, space="PSUM") as cp,
            tc.tile_pool(name="zp", bufs=2, space="PSUM") as zp,
            tc.tile_pool(name="rp", bufs=3) as rp,
        ):
            for J in range(2):
                ebf = []     # bf16 E tiles t=0,1
                epr = [None]  # fp8 E pair tiles, index p>=1
                for t in range(KJ[J]):
                    ps = sp.tile([P, 512], f32, tag="sc", name="sc")
                    for i in range(NPAIR):
                        nc.tensor.matmul(
                            ps[:],
                            kt8[i][:, :, P * t:P * (t + 1)],
                            qt8[i][:, :, 512 * J:512 * (J + 1)],
                            start=(i == 0), stop=(i == NPAIR - 1),
                            perf_mode=DR,
                        )
                    if t < NBF:
                        e = ebp.tile([P, 512], bt, tag="e", name=f"e{J}_{t}")
                        edst = e[:]
                        ebf.append(e)
                    else:
                        if t % 2 == 0:
                            epr.append(epp.tile([P, 2, 512], e4, tag="e8",
                                                name=f"e8_{J}_{t // 2}"))
                        edst = epr[t // 2][:, t % 2, :]
                    nc.scalar.activation(
                        edst, ps[:], mybir.ActivationFunctionType.Exp,
                        scale=EXP_SCALE)
                    for c in range(4):
                        j = 4 * J + c
                        if KAV[j] - 2 <= t <= KAV[j] - 1:
                            mslot = masks[:, 2 * j + (t - (KAV[j] - 2)), :]
                            if t < NBF:
                                dst = ebf[t][:, P * c:P * (c + 1)]
                            else:
                                dst = epr[t // 2][:, t % 2, P * c:P * (c + 1)]
                            nc.vector.tensor_mul(dst, dst, mslot)
                for c in range(4):
                    j = 4 * J + c
                    n = KAV[j]
                    ctx = cp.tile([P, D], f32, tag="ctx", name="ctx")
                    sm = zp.tile([P, 1], f32, tag="sm", name="sm")
                    last_pair = n // 2 - 1   # 0 -> no fp8 part
                    for t in range(NBF):
                        lhs = ebf[t][:, P * c:P * (c + 1)]
                        st = (t == 0)
                        sp_ = (t == NBF - 1) and (last_pair < 1)
                        nc.tensor.matmul(ctx[:, 0:512], lhs, vvb[t][:, 0:512],
                                         start=st, stop=sp_)
                        nc.tensor.matmul(ctx[:, 512:1024], lhs,
                                         vvb[t][:, 512:1024], start=st, stop=sp_)
                        nc.tensor.matmul(sm[:], lhs, ones[:], start=st, stop=sp_)
                    for p in range(1, last_pair + 1):
                        lhs = epr[p][:, :, P * c:P * (c + 1)]
                        sp_ = (p == last_pair)
                        nc.tensor.matmul(ctx[:, 0:512], lhs, vvp[p][:, :, 0:512],
                                         start=False, stop=sp_, perf_mode=DR)
                        nc.tensor.matmul(ctx[:, 512:1024], lhs,
                                         vvp[p][:, :, 512:1024],
                                         start=False, stop=sp_, perf_mode=DR)
                        nc.tensor.matmul(sm[:], lhs, ones8[:],
                                         start=False, stop=sp_, perf_mode=DR)
                    rc = rp.tile([P, 1], f32, tag="rc", name="rc")
                    nc.vector.reciprocal(rc[:], sm[:])
                    o = op.tile([P, D], bt, tag="o", name="o")
                    nc.vector.tensor_scalar_mul(o[:], ctx[:], rc[:])
                    nc.sync.dma_start(out_d[P * j:P * (j + 1), :], o[:])


_CACHE = {}


def _build(reps=1):
    nc = bacc.Bacc(
        "TRN2", target_bir_lowering=False, debug=False,
        enable_asserts=False, num_devices=N_CORES,
    )
    with tile.TileContext(nc) as tc:
        _emit(nc, tc, reps=reps)
    nc.compile()
    return nc


def _pack_pairs(m: np.ndarray) -> np.ndarray:
    """[1024, N] f32 -> [4, 128, 2, N] fp8, d = 256*i + 128*j + p."""
    n = m.shape[1]
    return np.ascontiguousarray(
        m.reshape(NPAIR, 2, P, n).transpose(0, 2, 1, 3)).astype(f8)


def _masks_for(h: int) -> np.ndarray:
    """[128, 16, 128] bf16; chunk j slots (2j, 2j+1) = masks for k-tiles
    KAV[j]-2 and KAV[j]-1. Scores layout [k=part, q=free]: allow k <= q."""
    tri = (np.arange(P)[:, None] <= np.arange(P)[None, :]).astype(np.float32)
    m = np.zeros((P, 16, P), dtype=np.float32)
    for j in range(8):
        g = GSEL[h][j]
        if g == KAV[j] - 1:
            m[:, 2 * j, :] = 1.0
            m[:, 2 * j + 1, :] = tri
        else:
            assert g == KAV[j] - 2
            m[:, 2 * j, :] = tri
            m[:, 2 * j + 1, :] = 0.0
    return m.astype(bf16)


def build_in_maps(x, W_query, W_key, W_value):
    wq8 = _pack_pairs(np.asarray(W_query, np.float32) * W8SCALE)
    wk8 = _pack_pairs(np.asarray(W_key, np.float32) * W8SCALE)
    wv8 = _pack_pairs(np.asarray(W_value, np.float32))
    wv = np.asarray(W_value, np.float32).astype(bf16)
    masks = [_masks_for(0), _masks_for(1)]
    in_maps = []
    for core in range(N_CORES):
        b, h = divmod(core, 2)
        xb = np.asarray(x[b], np.float32)
        xbt = np.ascontiguousarray(xb.T)               # [1024, 2048]
        qrows = np.concatenate([np.arange(P * g, P * (g + 1)) for g in GSEL[h]])
        in_maps.append({
            "xt8": _pack_pairs(xbt),
            "xtq8": _pack_pairs(np.ascontiguousarray(xb[qrows].T)),
            "wq8": wq8, "wk8": wk8, "wv8": wv8,
            "xt": np.ascontiguousarray(xbt[:, :NBF * P]).astype(bf16),
            "wv": wv,
            "masks": masks[h],
        })
    return in_maps


def assemble_out(results) -> np.ndarray:
    out = np.empty((B, S, D), dtype=np.float32)
    for core in range(N_CORES):
        b, h = divmod(core, 2)
        r = np.asarray(results[core]["out"], dtype=np.float32)
        for j, g in enumerate(GSEL[h]):
            out[b, P * g:P * (g + 1), :] = r[P * j:P * (j + 1), :]
    return out


def kernel(x, W_query, W_key, W_value):
    if "nc" not in _CACHE:
        _CACHE["nc"] = _build()
    nc = _CACHE["nc"]
    in_maps = build_in_maps(x, W_query, W_key, W_value)
    r = run_bass_kernel_spmd(nc, in_maps, core_ids=list(range(N_CORES)))
    return assemble_out(r.results)


if __name__ == "__main__":
    rng = np.random.default_rng(0)
    x = rng.standard_normal((B, S, D), dtype=np.float32)
    bound = 1.0 / np.sqrt(D)
    wq = rng.uniform(-bound, bound, (D, D)).astype(np.float32)
    wk = rng.uniform(-bound, bound, (D, D)).astype(np.float32)
    wv = rng.uniform(-bound, bound, (D, D)).astype(np.float32)
    o = kernel(x, wq, wk, wv)
    print("out", o.shape, o.dtype, float(np.abs(o).max()))


# revision 5
# speedup vs baseline: 3.9849x; 1.5166x over previous
"""Causal single-head attention [4, 2048, 1024] on 8 TRN2 NeuronCores.

Sharding: pure SPMD, no collectives. core = 2*b + h  (b = batch, h = query
zigzag half). Each core owns 8 query tiles of 128 rows, zigzag-interleaved so
causal work is balanced: h=0 -> global q128-tiles [0,2,4,6,9,11,13,15],
h=1 -> [1,3,5,7,8,10,12,14] (both sum to 68 causal k-tile visits).

v4: fp8 e4m3 DoubleRow everywhere except k-tiles 0-1 of the V/context path
(kept bf16: the first query tile's outputs are near-copies of single V rows
and dominate max-relative-error). E's fp8 quantization cancels through the
softmax denominator (summed from the same quantized E).

HW-measured matmul cost = fixed ~90ns + moving (213ns bf16 / 107ns fp8-DR
per 512 cols) + a weight load (~180-310ns) paid only when the stationary
operand CHANGES between consecutive matmuls. So every loop is ordered to
keep the stationary fixed across consecutive instructions, interleaving the
PSUM accumulation groups of the moving blocks instead:
  Q proj:  for (m,i): qb=0,1 share w-chunk      (2 psum groups in flight)
  K proj:  for (m,i): kb=0..3 share w-chunk     (4 groups)
  V proj:  for (k,i): fh=0,1 share x-chunk      (2 groups)
  scores:  for (t,i): J=0,1 share kt-chunk      (2 groups)
  context: lo/hi/sm share the E-chunk (already 3-way)
PSUM->SBUF copies alternate DVE/Activation; the final ctx*1/denom scale runs
on Activation (Copy with per-partition scale) to keep DVE off the critical
path.

Scaling: W_q8 = fp8(Wq*32), W_k8 = fp8(Wk*32); QT/KT PSUM (=32*q) copied to
fp8 unscaled; logits recovered in the exp activation with scale 2^-15.
Wv8 = fp8(Wv) unscaled (|Wv|<=1/32 sits in e4m3's 2^-6/subnormal range whose
fixed ~2^-10 step matches the scaled variant's top-binade error).

Causal masking: for context chunk j only k-tiles KAV[j]-2, KAV[j]-1 can
straddle the diagonal; each gets a [128,128] data-driven mask (ones/tri or
tri/zeros by zigzag parity), so one program serves both parities.
"""

import os
import sys

os.environ.setdefault("JAX_PLATFORMS", "axon")
for _p in (
    "/root/.axon_site",
    "/root/.axon_site/_ro/trn_rl_repo",
    "/root/.axon_site/_ro/pypackages",
    "/opt/trn_rl_repo",
):
    if os.path.isdir(_p) and _p not in sys.path:
        sys.path.append(_p)

import ml_dtypes
import numpy as np

import concourse.bass as bass  # noqa: F401  (import keeps bass registered)
import concourse.tile as tile
from concourse import bacc, mybir
from concourse.bass_utils import run_bass_kernel_spmd

bf16 = ml_dtypes.bfloat16
f8 = ml_dtypes.float8_e4m3

B, S, D = 4, 2048, 1024
P = 128
N_CORES = 8
W8SCALE = 32.0
EXP_SCALE = 1.0 / (W8SCALE * W8SCALE * 32.0)   # logits = psum * EXP_SCALE
NBF = 2                        # k128-tiles kept bf16 in the V/context path

GSEL = (
    [0, 2, 4, 6, 9, 11, 13, 15],   # h = 0
    [1, 3, 5, 7, 8, 10, 12, 14],   # h = 1
)
KJ = (8, 16)                   # scores k128-tile count per local q512 block
KAV = [2, 4, 6, 8, 10, 12, 14, 16]  # context k128-tile count per local q128
NPAIR = D // (2 * P)           # 4 contraction pair-tiles over d/f


def _emit(nc, tc, reps=1):
    bt = mybir.dt.bfloat16
    e4 = mybir.dt.float8e4

    xt8_d = nc.dram_tensor("xt8", [NPAIR, P, 2, S], e4, kind="ExternalInput").ap()
    xtq8_d = nc.dram_tensor("xtq8", [NPAIR, P, 2, D], e4, kind="ExternalInput").ap()
    wq8_d = nc.dram_tensor("wq8", [NPAIR, P, 2, D], e4, kind="ExternalInput").ap()
    wk8_d = nc.dram_tensor("wk8", [NPAIR, P, 2, D], e4, kind="ExternalInput").ap()
    wv8_d = nc.dram_tensor("wv8", [NPAIR, P, 2, D], e4, kind="ExternalInput").ap()
    xt_d = nc.dram_tensor("xt", [D, NBF * P], bt, kind="ExternalInput").ap()
    wv_d = nc.dram_tensor("wv", [D, D], bt, kind="ExternalInput").ap()
    mask_d = nc.dram_tensor("masks", [P, 16, P], bt, kind="ExternalInput").ap()
    out_d = nc.dram_tensor("out", [D, D], bt, kind="ExternalOutput").ap()

    for _rep in range(reps):
        _emit_once(nc, tc, xt8_d, xtq8_d, wq8_d, wk8_d, wv8_d, xt_d, wv_d,
                   mask_d, out_d)


def _emit_once(nc, tc, xt8_d, xtq8_d, wq8_d, wk8_d, wv8_d, xt_d, wv_d,
               mask_d, out_d):
    f32 = mybir.dt.float32
    bt = mybir.dt.bfloat16
    e4 = mybir.dt.float8e4
    ND = D // P                # 8
    DR = mybir.MatmulPerfMode.DoubleRow
    Exp = mybir.ActivationFunctionType.Exp
    Copy = mybir.ActivationFunctionType.Copy

    cp_alt = [0]

    def copy_out(dst, src):
        """alternate PSUM->SBUF copies between DVE and Activation"""
        cp_alt[0] ^= 1
        if cp_alt[0]:
            nc.vector.tensor_copy(dst, src)
        else:
            nc.scalar.activation(dst, src, Copy)

    with (
        tc.tile_pool(name="qp", bufs=NPAIR) as qp,
        tc.tile_pool(name="kp", bufs=NPAIR) as kp,
        tc.tile_pool(name="vbp", bufs=NBF) as vbp,
        tc.tile_pool(name="vpp", bufs=S // (2 * P) - 1) as vpp,
        tc.tile_pool(name="ebp", bufs=5) as ebp,
        tc.tile_pool(name="epp", bufs=11) as epp,
        tc.tile_pool(name="op", bufs=4) as op,
        tc.tile_pool(name="smallp", bufs=2) as smallp,
        tc.tile_pool(name="maskp", bufs=1) as maskp,
    ):
        ones = smallp.tile([P, 1], bt, tag="ones")
        nc.vector.memset(ones[:], 1.0)
        ones8 = smallp.tile([P, 2, 1], e4, tag="ones8")
        nc.vector.memset(ones8[:], 1.0)
        masks = maskp.tile([P, 16, P], bt, tag="masks")

        qt8 = [qp.tile([P, 2, D], e4, tag="qt", name=f"qt{i}") for i in range(NPAIR)]
        kt8 = [kp.tile([P, 2, S], e4, tag="kt", name=f"kt{i}") for i in range(NPAIR)]
        vvb = [vbp.tile([P, D], bt, tag="vb", name=f"vb{k}") for k in range(NBF)]
        # pair p holds k-tiles 2p, 2p+1 (p >= 1; tiles 0,1 are the bf16 vvb)
        vvp = [None] + [vpp.tile([P, 2, D], e4, tag="vp", name=f"vp{p}")
                        for p in range(1, S // (2 * P))]

        # ---- projections ----
        with (
            tc.tile_pool(name="wp", bufs=3 * NPAIR + ND) as wp,
            tc.tile_pool(name="xp", bufs=2 * NPAIR + ND) as xp,
            tc.tile_pool(name="pp", bufs=6, space="PSUM") as pp,
        ):
            # DMA issue order matters: the first matmul group needs wq8+xtq8.
            wq8t, xtq8t = [], []
            for i in range(NPAIR):
                t = wp.tile([P, 2, D], e4, tag="w", name=f"wq8{i}")
                nc.sync.dma_start(t[:], wq8_d[i])
                wq8t.append(t)
                t2 = xp.tile([P, 2, D], e4, tag="x", name=f"xtq8{i}")
                nc.sync.dma_start(t2[:], xtq8_d[i])
                xtq8t.append(t2)
            wk8t, xt8t = [], []
            for i in range(NPAIR):
                t = wp.tile([P, 2, D], e4, tag="w", name=f"wk8{i}")
                nc.sync.dma_start(t[:], wk8_d[i])
                wk8t.append(t)
                t2 = xp.tile([P, 2, S], e4, tag="x", name=f"xt8{i}")
                nc.sync.dma_start(t2[:], xt8_d[i])
                xt8t.append(t2)
            wv8t = []
            for i in range(NPAIR):
                t = wp.tile([P, 2, D], e4, tag="w", name=f"wv8{i}")
                nc.sync.dma_start(t[:], wv8_d[i])
                wv8t.append(t)
            xtt, wvt = [], []
            for di in range(ND):
                t = xp.tile([P, NBF * P], bt, tag="x", name=f"xt{di}")
                nc.sync.dma_start(t[:], xt_d[P * di:P * (di + 1), :])
                xtt.append(t)
                t2 = wp.tile([P, D], bt, tag="w", name=f"wv{di}")
                nc.sync.dma_start(t2[:], wv_d[P * di:P * (di + 1), :])
                wvt.append(t2)
            nc.sync.dma_start(masks[:], mask_d[:])

            # QT[f, q] (x32): per (m, i) the w-chunk stays stationary across
            # qb=0,1 (two interleaved PSUM groups)
            for m in range(ND):
                ps = [pp.tile([P, 512], f32, tag="ps", name="psq") for _ in range(2)]
                for i in range(NPAIR):
                    for qb in range(2):
                        nc.tensor.matmul(
                            ps[qb][:],
                            wq8t[i][:, :, P * m:P * (m + 1)],
                            xtq8t[i][:, :, 512 * qb:512 * (qb + 1)],
                            start=(i == 0), stop=(i == NPAIR - 1),
                            perf_mode=DR,
                        )
                for qb in range(2):
                    copy_out(qt8[m // 2][:, m % 2, 512 * qb:512 * (qb + 1)],
                             ps[qb][:])

            # KT[f, k] (x32): per (m, i) w-chunk stationary across kb=0..3
            for m in range(ND):
                ps = [pp.tile([P, 512], f32, tag="ps", name="psk") for _ in range(4)]
                for i in range(NPAIR):
                    for kb in range(S // 512):
                        nc.tensor.matmul(
                            ps[kb][:],
                            wk8t[i][:, :, P * m:P * (m + 1)],
                            xt8t[i][:, :, 512 * kb:512 * (kb + 1)],
                            start=(i == 0), stop=(i == NPAIR - 1),
                            perf_mode=DR,
                        )
                for kb in range(S // 512):
                    copy_out(kt8[m // 2][:, m % 2, 512 * kb:512 * (kb + 1)],
                             ps[kb][:])

            # V[k, f]: k-tiles 0..NBF-1 bf16, rest fp8 DoubleRow; per (k, i/di)
            # the x-chunk stays stationary across fh=0,1
            for k in range(S // P):
                ps = [pp.tile([P, 512], f32, tag="ps", name="psv") for _ in range(2)]
                if k < NBF:
                    for di in range(ND):
                        for fh in range(2):
                            nc.tensor.matmul(
                                ps[fh][:],
                                xtt[di][:, P * k:P * (k + 1)],
                                wvt[di][:, 512 * fh:512 * (fh + 1)],
                                start=(di == 0), stop=(di == ND - 1),
                            )
                    for fh in range(2):
                        copy_out(vvb[k][:, 512 * fh:512 * (fh + 1)], ps[fh][:])
                else:
                    for i in range(NPAIR):
                        for fh in range(2):
                            nc.tensor.matmul(
                                ps[fh][:],
                                xt8t[i][:, :, P * k:P * (k + 1)],
                                wv8t[i][:, :, 512 * fh:512 * (fh + 1)],
                                start=(i == 0), stop=(i == NPAIR - 1),
                                perf_mode=DR,
                            )
                    for fh in range(2):
                        copy_out(vvp[k // 2][:, k % 2, 512 * fh:512 * (fh + 1)],
                                 ps[fh][:])

        # ---- attention ----
        with (
            tc.tile_pool(name="sp", bufs=2, space="PSUM") as sp,
            tc.tile_pool(name="cp", bufs=2, space="PSUM") as cp,
            tc.tile_pool(name="zp", bufs=2, space="PSUM") as zp,
            tc.tile_pool(name="rp", bufs=3) as rp,
        ):
            # scores for BOTH q-blocks in one k-sweep: per (t, i) the kt-chunk
            # stays stationary across J (two interleaved PSUM groups)
            ebf = {}   # (J, t) -> bf16 E tile, t < NBF
            epr = {}   # (J, p) -> fp8 E pair tile, p >= 1
            for t in range(KJ[1]):
                Js = [J for J in range(2) if t < KJ[J]]
                ps = {J: sp.tile([P, 512], f32, tag="sc", name="sc") for J in Js}
                for i in range(NPAIR):
                    for J in Js:
                        nc.tensor.matmul(
                            ps[J][:],
                            kt8[i][:, :, P * t:P * (t + 1)],
                            qt8[i][:, :, 512 * J:512 * (J + 1)],
                            start=(i == 0), stop=(i == NPAIR - 1),
                            perf_mode=DR,
                        )
                for J in Js:
                    if t < NBF:
                        e = ebp.tile([P, 512], bt, tag="e", name=f"e{J}_{t}")
                        ebf[(J, t)] = e
                        edst = e[:]
                    else:
                        if t % 2 == 0:
                            epr[(J, t // 2)] = epp.tile(
                                [P, 2, 512], e4, tag="e8", name=f"e8_{J}_{t // 2}")
                        edst = epr[(J, t // 2)][:, t % 2, :]
                    nc.scalar.activation(edst, ps[J][:], Exp, scale=EXP_SCALE)
                    for c in range(4):
                        j = 4 * J + c
                        if KAV[j] - 2 <= t <= KAV[j] - 1:
                            mslot = masks[:, 2 * j + (t - (KAV[j] - 2)), :]
                            if t < NBF:
                                dst = ebf[(J, t)][:, P * c:P * (c + 1)]
                            else:
                                dst = epr[(J, t // 2)][:, t % 2, P * c:P * (c + 1)]
                            nc.vector.tensor_mul(dst, dst, mslot)

            for J in range(2):
                for c in range(4):
                    j = 4 * J + c
                    n = KAV[j]
                    ctx = cp.tile([P, D], f32, tag="ctx", name="ctx")
                    sm = zp.tile([P, 1], f32, tag="sm", name="sm")
                    last_pair = n // 2 - 1   # 0 -> no fp8 part
                    for t in range(NBF):
                        lhs = ebf[(J, t)][:, P * c:P * (c + 1)]
                        st = (t == 0)
                        sp_ = (t == NBF - 1) and (last_pair < 1)
                        nc.tensor.matmul(ctx[:, 0:512], lhs, vvb[t][:, 0:512],
                                         start=st, stop=sp_)
                        nc.tensor.matmul(ctx[:, 512:1024], lhs,
                                         vvb[t][:, 512:1024], start=st, stop=sp_)
                        nc.tensor.matmul(sm[:], lhs, ones[:], start=st, stop=sp_)
                    for p in range(1, last_pair + 1):
                        lhs = epr[(J, p)][:, :, P * c:P * (c + 1)]
                        sp_ = (p == last_pair)
                        nc.tensor.matmul(ctx[:, 0:512], lhs, vvp[p][:, :, 0:512],
                                         start=False, stop=sp_, perf_mode=DR)
                        nc.tensor.matmul(ctx[:, 512:1024], lhs,
                                         vvp[p][:, :, 512:1024],
                                         start=False, stop=sp_, perf_mode=DR)
                        nc.tensor.matmul(sm[:], lhs, ones8[:],
                                         start=False, stop=sp_, perf_mode=DR)
                    rc = rp.tile([P, 1], f32, tag="rc", name="rc")
                    nc.vector.reciprocal(rc[:], sm[:])
                    o = op.tile([P, D], bt, tag="o", name="o")
                    nc.scalar.activation(o[:], ctx[:], Copy, scale=rc[:])
                    nc.sync.dma_start(out_d[P * j:P * (j + 1), :], o[:])


_CACHE = {}


def _build(reps=1):
    nc = bacc.Bacc(
        "TRN2", target_bir_lowering=False, debug=False,
        enable_asserts=False, num_devices=N_CORES,
    )
    with tile.TileContext(nc) as tc:
        _emit(nc, tc, reps=reps)
    nc.compile()
    return nc


def _pack_pairs(m: np.ndarray) -> np.ndarray:
    """[1024, N] f32 -> [4, 128, 2, N] fp8, d = 256*i + 128*j + p."""
    n = m.shape[1]
    return np.ascontiguousarray(
        m.reshape(NPAIR, 2, P, n).transpose(0, 2, 1, 3)).astype(f8)


def _masks_for(h: int) -> np.ndarray:
    """[128, 16, 128] bf16; chunk j slots (2j, 2j+1) = masks for k-tiles
    KAV[j]-2 and KAV[j]-1. Scores layout [k=part, q=free]: allow k <= q."""
    tri = (np.arange(P)[:, None] <= np.arange(P)[None, :]).astype(np.float32)
    m = np.zeros((P, 16, P), dtype=np.float32)
    for j in range(8):
        g = GSEL[h][j]
        if g == KAV[j] - 1:
            m[:, 2 * j, :] = 1.0
            m[:, 2 * j + 1, :] = tri
        else:
            assert g == KAV[j] - 2
            m[:, 2 * j, :] = tri
            m[:, 2 * j + 1, :] = 0.0
    return m.astype(bf16)


def build_in_maps(x, W_query, W_key, W_value):
    wq8 = _pack_pairs(np.asarray(W_query, np.float32) * W8SCALE)
    wk8 = _pack_pairs(np.asarray(W_key, np.float32) * W8SCALE)
    wv8 = _pack_pairs(np.asarray(W_value, np.float32))
    wv = np.asarray(W_value, np.float32).astype(bf16)
    masks = [_masks_for(0), _masks_for(1)]
    in_maps = []
    for core in range(N_CORES):
        b, h = divmod(core, 2)
        xb = np.asarray(x[b], np.float32)
        xbt = np.ascontiguousarray(xb.T)               # [1024, 2048]
        qrows = np.concatenate([np.arange(P * g, P * (g + 1)) for g in GSEL[h]])
        in_maps.append({
            "xt8": _pack_pairs(xbt),
            "xtq8": _pack_pairs(np.ascontiguousarray(xb[qrows].T)),
            "wq8": wq8, "wk8": wk8, "wv8": wv8,
            "xt": np.ascontiguousarray(xbt[:, :NBF * P]).astype(bf16),
            "wv": wv,
            "masks": masks[h],
        })
    return in_maps


def assemble_out(results) -> np.ndarray:
    out = np.empty((B, S, D), dtype=np.float32)
    for core in range(N_CORES):
        b, h = divmod(core, 2)
        r = np.asarray(results[core]["out"], dtype=np.float32)
        for j, g in enumerate(GSEL[h]):
            out[b, P * g:P * (g + 1), :] = r[P * j:P * (j + 1), :]
    return out


def kernel(x, W_query, W_key, W_value):
    if "nc" not in _CACHE:
        _CACHE["nc"] = _build()
    nc = _CACHE["nc"]
    in_maps = build_in_maps(x, W_query, W_key, W_value)
    r = run_bass_kernel_spmd(nc, in_maps, core_ids=list(range(N_CORES)))
    return assemble_out(r.results)


if __name__ == "__main__":
    rng = np.random.default_rng(0)
    x = rng.standard_normal((B, S, D), dtype=np.float32)
    bound = 1.0 / np.sqrt(D)
    wq = rng.uniform(-bound, bound, (D, D)).astype(np.float32)
    wk = rng.uniform(-bound, bound, (D, D)).astype(np.float32)
    wv = rng.uniform(-bound, bound, (D, D)).astype(np.float32)
    o = kernel(x, wq, wk, wv)
    print("out", o.shape, o.dtype, float(np.abs(o).max()))
